# revision 1
# baseline (speedup 1.0000x reference)
"""Graycode encoder kernel for Trainium2 (Bass/Tile), 8-core data-parallel.

Input  X: (8, 65536, 3) float32 (full).
Output:   (8, 65536, 96) int32 (full).

Per coordinate dim d (each 32 output channels):
  raw  = round(x)            (RNE, matches jnp.round)
  sign = raw > 0             -> channel 32*d
  g    = |raw| ^ (|raw| >> 1)
  bit k of g (k=0..30)       -> channel 32*d + 1 + k

Sharding: batch axis across the 8 cores (core b handles X[b]).

Per-core layout: points n = p*512 + t with p in [0,128) the SBUF partition
and t in [0,512). Processed in chunks of TC t-values. Bit-plane extraction
uses one DVE tensor_scalar per bit k, covering all three coordinate dims in
one instruction via strided access patterns:
    in : g[p, t*3 + d]                 (dims [t, d])
    out: out[p, t*96 + 32*d + 1 + k]   (dims [t, d])
HW f32->int32 conversion rounds to nearest-even (verified on device), so
round() is a plain dtype-converting copy, and sign = (x > 0.5) exactly.
"""

import numpy as np

import concourse.tile as tile
from concourse import bacc, mybir
from concourse.bass_utils import run_bass_kernel_spmd

A = mybir.AluOpType
ACTF = mybir.ActivationFunctionType
F32, I32 = mybir.dt.float32, mybir.dt.int32

B, N, D = 8, 65536, 3
P = 128            # SBUF partitions
T = N // P         # 512 t-values per partition
TC = 128           # t-chunk size
NCHUNK = T // TC
NBITS = 31         # gray-code bits per dim (channels 1+k)
CH = 96            # output channels

_CACHE = {}


def _stt_int(eng, out, in0, scalar, in1, op0, op1):
    """scalar_tensor_tensor with an int32 immediate: out = (in0 op0 s) op1 in1."""
    return eng.add_instruction(
        mybir.InstTensorScalarPtr(
            name=eng.bass.get_next_instruction_name(),
            is_scalar_tensor_tensor=True,
            op0=op0,
            op1=op1,
            ins=[eng.lower_ap(in0),
                 mybir.ImmediateValue(dtype=I32, value=scalar),
                 eng.lower_ap(in1)],
            outs=[eng.lower_ap(out)],
        )
    )


def _build():
    if "nc" in _CACHE:
        return _CACHE["nc"]

    nc = bacc.Bacc("TRN2", target_bir_lowering=False, debug=False, num_devices=B)
    x = nc.dram_tensor("x", [N, D], F32, kind="ExternalInput").ap()
    out = nc.dram_tensor("out", [N, CH], I32, kind="ExternalOutput").ap()

    x_r = x.rearrange("(p t) d -> p t d", p=P)        # [128, 512, 3]
    out_r = out.rearrange("(p t) j -> p t j", p=P)    # [128, 512, 96]

    with tile.TileContext(nc) as tc:
        with (
            tc.tile_pool(name="pin", bufs=2) as pin,
            tc.tile_pool(name="ptmp", bufs=2) as ptmp,
            tc.tile_pool(name="pout", bufs=2) as pout,
        ):
            for c in range(NCHUNK):
                t0 = c * TC
                tin = pin.tile([P, TC * D], F32)
                nc.sync.dma_start(
                    tin[:].rearrange("p (t d) -> p t d", d=D),
                    x_r[:, t0:t0 + TC, :],
                )
                tin_r = tin[:].rearrange("p (t d) -> p t d", d=D)

                # |x| on ACT engine (f32 -> f32)
                absf = ptmp.tile([P, TC * D], F32, tag="absf")
                nc.scalar.activation(absf[:], tin[:], ACTF.Abs)

                # vi = int32(round(|x|)) -- RNE conversion in the copy
                vi = ptmp.tile([P, TC * D], I32, tag="vi")
                nc.vector.tensor_copy(vi[:], absf[:])

                # g = (vi >> 1) ^ vi
                g = ptmp.tile([P, TC * D], I32, tag="g")
                _stt_int(nc.vector, g[:], vi[:], 1, vi[:],
                         A.logical_shift_right, A.bitwise_xor)
                g_r = g[:].rearrange("p (t d) -> p t d", d=D)

                tout = pout.tile([P, TC * CH], I32)
                # view as [p, t, d, k] with channel j = 32*d + k
                tout_r = tout[:].rearrange("p (t d k) -> p t d k", d=D, k=32)

                # sign channels (k=0): round(x) > 0  <=>  x > 0.5
                nc.vector.tensor_scalar(tout_r[:, :, :, 0], tin_r, 0.5, None,
                                        A.is_gt)

                # bit planes: one op per k covers all 3 dims
                for k in range(NBITS):
                    nc.vector.tensor_scalar(tout_r[:, :, :, 1 + k], g_r, k, 1,
                                            A.logical_shift_right,
                                            A.bitwise_and)

                nc.sync.dma_start(
                    out_r[:, t0:t0 + TC, :],
                    tout[:].rearrange("p (t j) -> p t j", j=CH),
                )

    nc.compile()
    _CACHE["nc"] = nc
    return nc


def kernel(X, **run_kwargs):
    nc = _build()
    X = np.asarray(X, dtype=np.float32)
    assert X.shape == (B, N, D), X.shape
    in_maps = [{"x": np.ascontiguousarray(X[b])} for b in range(B)]
    res = run_bass_kernel_spmd(nc, in_maps, core_ids=list(range(B)), **run_kwargs)
    out = np.stack([r["out"] for r in res.results], axis=0)
    if run_kwargs:
        kernel.last_result = res
    return out
